# revision 42
# baseline (speedup 1.0000x reference)
"""Trainium2 Bass kernel for multi-head attention (dense transformer block).

Reference computation (per batch element):
    qkv = x @ w_qkv                      # [N, 3C]
    q, k, v = split heads (H=12, HD=64); q *= HD**-0.5
    out = softmax(q k^T) v               # full [N, N] scores
    out = merge_heads(out) @ w_proj + b_proj

Distribution: pure data parallel over the batch dim — B=8 batch elements,
8 NeuronCores, one element per core.  Weights are replicated.  No
collectives are needed; each core computes its full [2048, 768] output.

Per-core compute strategy (all matmuls fp16, fp32 PSUM accumulation;
fp16 keeps 10 mantissa bits vs bf16's 7, tightening the error budget):
  * x is cast f32->fp16 by a SWDGE DMA into a DRAM scratch, then DMA-xbar
    transposed into SBUF as xT [768, 2048] in 512-token blocks.
  * qkT = w_qk^T @ x^T -> [1536, 2048]: q/k for a head PAIR live in one
    128-partition tile (head A on partitions 0-63, head B on 64-127).
  * scoresT[m, n] = kT^T qT per head: keys on partitions, queries on the
    free dim; one [128, 1024] PSUM tile per (pair, key-tile) covers both
    heads x 512 queries.
  * exp splits across two engines: most tiles on ScalarE (exp activation,
    softmax 1/8 scale folded into the free affine; no max subtraction -
    scaled scores are ~N(0,1)).  Key tiles in DVE_EXP_MS instead run a
    Schraudolph integer exp on VectorE: one tensor_scalar computes
    round(s*A + B) into an int16 alias of the fp16 tile, which IS the
    fp16 bit pattern of 2^(s*SCALE/ln2) (+-3% on those weights; washes
    out across the 2048-key softmax).
  * attnV runs in the natural (queries-on-partition) orientation with at
    as the stationary operand: out[128q, 64] = at_slice^T @ v_h, so each
    accumulation step streams only 64 columns - half the PE cost of the
    transposed orientation.  A companion 1-column ones matmul per slice
    accumulates the softmax denominator at ~zero PE cost.
  * normalization is a per-partition (per-query) reciprocal multiply -
    one [128,1] reciprocal + one tensor_scalar per (head, query-slice).
  * out_nat [512, 768] chunks are PE-transposed (identity matmul) into
    outT as soon as each head pair is evicted, feeding the final
    projection outT^T @ w_proj + b, interleaved into the next chunk.
"""

import os

import numpy as np

import concourse.bass as bass
import concourse.mybir as mybir
from concourse import bacc, bass_utils
from concourse.tile import TileContext

F32 = mybir.dt.float32
FP16 = mybir.dt.float16
I16 = mybir.dt.int16
AF = mybir.ActivationFunctionType

B, N, C = 8, 2048, 768
H, HD = 12, 64
SCALE = HD ** -0.5  # folded into the exp activation
P = 128
NT = N // P          # 16 key tiles
CT = C // P          # 6 feature tiles
NCHUNK = 4           # query chunks of 512
QW = N // NCHUNK     # 512
PAIRS = H // 2       # 6 head pairs
SKEW = 5             # scores/exp run this many key-tiles ahead of attnV

# Schraudolph integer exp: fp16 bits of 2^(s*SCALE/ln2) ~= round(s*A + B)
EXP_A = SCALE * 1024.0 / np.log(2.0)
EXP_B = 15.0 * 1024.0 - 44.0        # mid-centered: +-3% rel err
# key tiles exp'd on VectorE, per chunk: chunk 0 is PE/DVE-heavy (qkv
# projections + evictions) so ScalarE takes more of the exp there
DVE_EXP_MS = {0: (2, 6, 10, 14), 1: (2, 4, 7, 9, 12, 14),
              2: (2, 4, 7, 9, 12, 14), 3: (2, 4, 7, 9, 12, 14)}


def build_nc() -> bass.Bass:
    nc = bacc.Bacc(None)
    x = nc.declare_dram_parameter("x", [N, C], F32, isOutput=False)
    w_qkv = nc.declare_dram_parameter("w_qkv", [C, 3 * C], F32, isOutput=False)
    w_proj = nc.declare_dram_parameter("w_proj", [C, C], F32, isOutput=False)
    b_proj = nc.declare_dram_parameter("b_proj", [C], F32, isOutput=False)
    out = nc.declare_dram_parameter("out", [N, C], F32, isOutput=True)

    with TileContext(nc) as tc:
        with (
            tc.tile_pool(name="const", bufs=1) as cpool,
            tc.tile_pool(name="dram", bufs=1, space="DRAM") as dpool,
            tc.tile_pool(name="at", bufs=8) as at_pool,
            tc.tile_pool(name="rcp", bufs=8) as rcp_pool,
            tc.tile_pool(name="onat", bufs=2) as onat_pool,
            tc.tile_pool(name="fin", bufs=2) as fin_pool,
            tc.tile_pool(name="psc", bufs=2, space="PSUM") as psum_sc,
            tc.tile_pool(name="pqk", bufs=2, space="PSUM") as psum_qk,
            tc.tile_pool(name="pav", bufs=1, space="PSUM") as psum_av,
            tc.tile_pool(name="pden", bufs=1, space="PSUM") as psum_den,
        ):
            # ---- persistent SBUF tensors -------------------------------
            w_qkv_sb = cpool.tile([P, CT, 3 * C], FP16, tag="wqkv")
            wproj_sb = cpool.tile([P, CT, C], FP16, tag="wproj")
            b_bc = cpool.tile([P, C], F32, tag="bias")  # bias bcast to 128 rows
            onescol = cpool.tile([P, 1], FP16, tag="onescol")
            ident = cpool.tile([P, P], FP16, tag="ident")
            # 24 separate xT blocks (ct, token-chunk): independent tiles let
            # the 24 DMA transposes run without false write-order deps
            xT = {
                (ct, tch): cpool.tile(
                    [P, QW], FP16, tag=f"xT{ct}_{tch}", name=f"xT{ct}_{tch}"
                )
                for ct in range(CT)
                for tch in range(NCHUNK)
            }
            qkT = cpool.tile([P, 12, N], FP16, tag="qkT")  # q(6 pairs)+k(6)
            v4 = cpool.tile([P, NT, C], FP16, tag="v4")
            outT = cpool.tile([P, CT, N], FP16, tag="outT")

            # ---- phase 0: load + cast + transpose ----------------------
            # per-token-chunk contiguous f32->fp16 casts (cheap flat APs),
            # spread across engine DMA queues: chunk 0 on Pool (first), the
            # rest on ScalarE (idle until the first exp); the 24 xbar
            # transposes alternate between the SP and DVE queues
            nc.any.memset(onescol[:], 1.0)
            nc.any.memset(ident[:], 1.0)
            # identity: keep 1.0 where (p - col) == 0, else 0
            nc.gpsimd.affine_select(
                ident[:], ident[:], pattern=[[-1, P]],
                compare_op=mybir.AluOpType.is_equal, fill=0.0,
                base=0, channel_multiplier=1,
            )
            wq3 = w_qkv.rearrange("(o p) j -> p o j", p=P)
            wp3 = w_proj.rearrange("(o p) j -> p o j", p=P)
            # all casting DMAs must go via gpsimd; keep their APs 2-D
            # (column-split, non-contiguous) so each is charged as a short
            # per-row transfer.  Load order on the Pool queue: x chunk 0,
            # the k-weight block (first scores need it), remaining x chunks,
            # then q weights, v weights, w_proj.
            xblks = []
            for tch in range(NCHUNK):
                xblk = dpool.tile([QW, C], FP16, name=f"xf{tch}", tag=f"xf{tch}")
                xblks.append(xblk)

            def cast_x(tch):
                tsl = slice(tch * QW, (tch + 1) * QW)
                for half in range(2):
                    csl = slice(half * 384, (half + 1) * 384)
                    nc.gpsimd.dma_start(
                        out=xblks[tch][:, csl], in_=x[tsl, csl]
                    )

            def emit_transposes(tch):
                for ct in range(CT):
                    csl = slice(ct * P, (ct + 1) * P)
                    eng = nc.sync if (tch * CT + ct) % 2 == 0 else nc.scalar
                    eng.dma_start_transpose(
                        xT[(ct, tch)][:], xblks[tch][:, csl]
                    )

            def load_w_block(blk):  # 0=q, 1=k, 2=v columns of w_qkv
                jsl = slice(blk * C, (blk + 1) * C)
                for ct in range(CT):
                    nc.gpsimd.dma_start(
                        out=w_qkv_sb[:, ct, jsl], in_=wq3[:, ct, jsl]
                    )

            cast_x(0)
            emit_transposes(0)
            load_w_block(1)  # k weights
            load_w_block(0)  # q weights
            for tch in range(1, NCHUNK):
                cast_x(tch)
                emit_transposes(tch)
            load_w_block(2)  # v weights
            for ct in range(CT):
                nc.gpsimd.dma_start(out=wproj_sb[:, ct, :], in_=wp3[:, ct, :])
            nc.sync.dma_start(
                out=b_bc[:], in_=b_proj[None, :].to_broadcast((P, C))
            )

            # ---- qkv projection groups --------------------------------
            # qkT[j, n] for j in [0, 1536): q rows 0-767, k rows 768-1535
            def emit_qk_group(jt: int, c4: int):
                ps = psum_qk.tile([P, 512], F32, tag="qk", name="qkps")
                for ct in range(CT):
                    nc.tensor.matmul(
                        ps[:],
                        lhsT=w_qkv_sb[:, ct, jt * P : (jt + 1) * P],
                        rhs=xT[(ct, c4)][:],
                        start=(ct == 0),
                        stop=(ct == CT - 1),
                    )
                nc.vector.tensor_copy(
                    out=qkT[:, jt, c4 * QW : (c4 + 1) * QW], in_=ps[:]
                )

            # v natural layout: v[n, e] = sum_c x[n, c] w_qkv[c, 1536 + e]
            def emit_v_group(nt: int, hq: int):
                ps = psum_qk.tile([P, 512], F32, tag="qk", name="vps")
                eo = hq * 256
                for ct in range(CT):
                    nc.tensor.matmul(
                        ps[:, 0:256],
                        lhsT=xT[(ct, nt // 4)][:, (nt % 4) * P : (nt % 4 + 1) * P],
                        rhs=w_qkv_sb[:, ct, 2 * C + eo : 2 * C + eo + 256],
                        start=(ct == 0),
                        stop=(ct == CT - 1),
                    )
                nc.vector.tensor_copy(
                    out=v4[:, nt, eo : eo + 256], in_=ps[:, 0:256]
                )

            # ---- transposed eviction + final projection ----------------
            def emit_transpose_round(c_done: int, ct: int, onat):
                """outT[:, ct, c*512:+512] = out_nat chunk columns ct, via PE."""
                ps = psum_qk.tile([P, 512], FP16, tag="qk", name="trps")
                for qs in range(4):
                    nc.tensor.transpose(
                        ps[:, qs * P : (qs + 1) * P],
                        onat[:, qs, ct * P : (ct + 1) * P],
                        ident[:],
                    )
                nc.vector.tensor_copy(
                    out=outT[:, ct, c_done * QW : (c_done + 1) * QW], in_=ps[:]
                )

            def emit_proj_group(nt: int, eo: int, ew: int):
                """final[nt-tile, eo:eo+ew] = outT^T w_proj + b."""
                ps = psum_qk.tile([P, 512], F32, tag="qk", name="projps")
                for ct in range(CT):
                    nc.tensor.matmul(
                        ps[:, 0:ew],
                        lhsT=outT[:, ct, nt * P : (nt + 1) * P],
                        rhs=wproj_sb[:, ct, eo : eo + ew],
                        start=(ct == 0),
                        stop=(ct == CT - 1),
                    )
                fs = fin_pool.tile([P, 512], F32, tag="fin")
                nc.vector.tensor_tensor(
                    fs[:, 0:ew], ps[:, 0:ew], b_bc[:, eo : eo + ew],
                    mybir.AluOpType.add,
                )
                nc.sync.dma_start(
                    out=out[nt * P : (nt + 1) * P, eo : eo + ew], in_=fs[:, 0:ew]
                )

            def emit_proj_slot(c_done: int, slot: int):
                nt = c_done * 4 + slot // 2
                eo, ew = ((0, 512), (512, 256))[slot % 2]
                emit_proj_group(nt, eo, ew)

            # chunk-0 JIT schedule: (pair, scores-m) -> qk group
            c0_qk = {}
            for p0 in range(PAIRS):
                if p0 == 0:
                    c0_qk.update({(0, 2): (6, 1), (0, 5): (6, 2), (0, 8): (6, 3),
                                  (0, 10): (7, 0), (0, 11): (7, 1), (0, 12): (7, 2),
                                  (0, 13): (7, 3), (0, 14): (1, 0)})
                elif p0 < PAIRS - 1:
                    c0_qk.update({
                        (p0, 1): (7 + p0, 0), (p0, 4): (7 + p0, 1),
                        (p0, 7): (7 + p0, 2), (p0, 10): (7 + p0, 3),
                        (p0, 13): (1 + p0, 0),
                    })
                    if p0 == 1:
                        c0_qk[(p0, 14)] = (0, 1)
            # chunk 1 also absorbs chunk 0's deferred transposes (chunk 0's
            # own psum ring is saturated by the qkv projection storm)
            late_slots_c1 = {}
            for p0 in range(3):
                late_slots_c1[(p0, 5)] = ("t", 2 * p0)
                late_slots_c1[(p0, 11)] = ("t", 2 * p0 + 1)
            for p0 in range(3, PAIRS):
                late_slots_c1[(p0, 5)] = ("p", 2 * (p0 - 3))
                late_slots_c1[(p0, 11)] = ("p", 2 * (p0 - 3) + 1)
            late_slots_c1[(4, 13)] = ("p", 6)
            late_slots_c1[(5, 13)] = ("p", 7)
            # chunks >=2: proj of chunk c-1 spread over pairs 0-3
            late_slots = {}
            for p0 in range(4):
                late_slots[(p0, 5)] = ("p", 2 * p0)
                late_slots[(p0, 11)] = ("p", 2 * p0 + 1)

            # upfront: only what (c=0, pair=0, m<4) needs
            emit_qk_group(6, 0)   # kT pair 0, keys 0-511
            emit_qk_group(0, 0)   # qT pair 0, queries 0-511

            onat_prev = [None]  # chunk 0's onat, for its deferred transposes

            # ---- phase 2+3: attention + projection ---------------------
            for c in range(NCHUNK):
                qsl = slice(c * QW, (c + 1) * QW)
                onat = onat_pool.tile([P, 4, C], FP16, tag="onat", name="onat")
                for pair in range(PAIRS):
                    av = psum_av.tile([P, 512], F32, tag="av", name="av")
                    den = psum_den.tile([P, 8], F32, tag="den", name="den")
                    ats = [None] * NT
                    for mm_i in range(NT + SKEW):
                        # ---- scores + exp stage (runs SKEW ahead) ------
                        m = mm_i
                        if m < NT:
                            if c == 0:
                                if pair % 2 == 0:
                                    emit_v_group(m, pair // 2)
                                if (pair, m) in c0_qk:
                                    emit_qk_group(*c0_qk[(pair, m)])
                            else:
                                slots = late_slots_c1 if c == 1 else late_slots
                                if (pair, m) in slots:
                                    kind, s = slots[(pair, m)]
                                    if kind == "t":
                                        emit_transpose_round(0, s, onat_prev[0])
                                    else:
                                        emit_proj_slot(c - 1, s)
                                if c == 1 and m == 8 and pair < PAIRS - 1:
                                    emit_qk_group(pair + 1, 1)
                                if c < NCHUNK - 1 and m == 14:
                                    emit_qk_group(pair, c + 1)
                            msl = slice(m * P, (m + 1) * P)
                            sc = psum_sc.tile([P, 1024], F32, tag="sc")
                            nc.tensor.matmul(
                                sc[:, 0:QW],
                                lhsT=qkT[0:64, 6 + pair, msl],
                                rhs=qkT[0:64, pair, qsl],
                                start=True,
                                stop=True,
                            )
                            nc.tensor.matmul(
                                sc[:, QW : 2 * QW],
                                lhsT=qkT[64:128, 6 + pair, msl],
                                rhs=qkT[64:128, pair, qsl],
                                start=True,
                                stop=True,
                            )
                            at = at_pool.tile([P, 1024], FP16, tag="at")
                            ats[m] = at
                            if m in DVE_EXP_MS[c]:
                                # Schraudolph: int16 bits = round(s*A + B) are
                                # the fp16 pattern of exp(s*SCALE) (+-3%)
                                nc.vector.tensor_scalar(
                                    at.bitcast(I16)[:],
                                    sc[:],
                                    EXP_A,
                                    EXP_B,
                                    mybir.AluOpType.mult,
                                    mybir.AluOpType.add,
                                )
                            else:
                                nc.scalar.activation(
                                    at[:], sc[:], AF.Exp, scale=SCALE
                                )
                        # ---- attnV + denominator stage -----------------
                        if mm_i >= SKEW:
                            m = mm_i - SKEW
                            at = ats[m]
                            for hh in range(2):
                                h = 2 * pair + hh
                                for qs in range(4):
                                    lsl = at[:, hh * QW + qs * P : hh * QW + (qs + 1) * P]
                                    # start=True zeroes the whole 2KB bank
                                    # region, so only the bank's FIRST
                                    # sub-column may set it; later columns
                                    # overwrite-on-first-touch via the
                                    # pending-zero bytes it marked
                                    nc.tensor.matmul(
                                        av[:, (hh * 4 + qs) * 64 : (hh * 4 + qs + 1) * 64],
                                        lhsT=lsl,
                                        rhs=v4[:, m, h * 64 : (h + 1) * 64],
                                        start=(m == 0 and hh * 4 + qs == 0),
                                        stop=(m == NT - 1),
                                        skip_group_check=(hh * 4 + qs != 0),
                                    )
                                    nc.tensor.matmul(
                                        den[:, hh * 4 + qs : hh * 4 + qs + 1],
                                        lhsT=lsl,
                                        rhs=onescol[:],
                                        start=(m == 0 and hh * 4 + qs == 0),
                                        stop=(m == NT - 1),
                                        skip_group_check=(hh * 4 + qs != 0),
                                    )

                    # ---- normalize + evict this (pair, chunk) ----------
                    # reciprocal on DVE, per-partition-scaled Copy on ScalarE
                    for hh in range(2):
                        h = 2 * pair + hh
                        for qs in range(4):
                            rcp = rcp_pool.tile([P, 1], F32, tag="rcp")
                            nc.vector.reciprocal(
                                rcp[:], den[:, hh * 4 + qs : hh * 4 + qs + 1]
                            )
                            nc.scalar.activation(
                                onat[:, qs, h * 64 : (h + 1) * 64],
                                av[:, (hh * 4 + qs) * 64 : (hh * 4 + qs + 1) * 64],
                                AF.Copy,
                                scale=rcp[:],
                            )
                    # transpose round `pair` only reads this pair's columns
                    # of onat (heads 2p, 2p+1 = feature block ct=pair), so it
                    # can run as soon as this pair is evicted (chunk 0's
                    # rounds are deferred into chunk 1 instead)
                    if c == 0:
                        onat_prev[0] = onat
                    else:
                        emit_transpose_round(c, pair, onat)
            # tail: proj for the last chunk
            for slot in range(8):
                emit_proj_slot(NCHUNK - 1, slot)

    nc.compile()
    return nc


_NC_CACHE: list = []


def _get_nc() -> bass.Bass:
    if not _NC_CACHE:
        _NC_CACHE.append(build_nc())
    return _NC_CACHE[0]


def run(inputs: dict, trace: bool = False):
    """Run on 8 NeuronCores.  Returns (out [B,N,C] f32, exec_time_ns|None)."""
    nc = _get_nc()
    x = np.ascontiguousarray(np.asarray(inputs["x"], dtype=np.float32))
    w_qkv = np.ascontiguousarray(np.asarray(inputs["w_qkv"], dtype=np.float32))
    w_proj = np.ascontiguousarray(np.asarray(inputs["w_proj"], dtype=np.float32))
    b_proj = np.ascontiguousarray(np.asarray(inputs["b_proj"], dtype=np.float32))
    in_maps = [
        {"x": x[i], "w_qkv": w_qkv, "w_proj": w_proj, "b_proj": b_proj}
        for i in range(B)
    ]
    try:
        res = bass_utils.run_bass_kernel_spmd(
            nc, in_maps, core_ids=list(range(B)), trace=trace
        )
    except ModuleNotFoundError:
        # NTFF profile hook unavailable in this image; run without trace
        res = bass_utils.run_bass_kernel_spmd(
            nc, in_maps, core_ids=list(range(B)), trace=False
        )
    out = np.stack([res.results[i]["out"] for i in range(B)], axis=0)
    return out.astype(np.float32), res.exec_time_ns


def kernel(x, w_qkv, w_proj, b_proj):
    trace = os.environ.get("BASS_KERNEL_TRACE", "0") == "1"
    out, _ = run(
        {"x": x, "w_qkv": w_qkv, "w_proj": w_proj, "b_proj": b_proj}, trace=trace
    )
    return out


# revision 43
# speedup vs baseline: 1.0389x; 1.0389x over previous
"""Trainium2 Bass kernel for multi-head attention (dense transformer block).

Reference computation (per batch element):
    qkv = x @ w_qkv                      # [N, 3C]
    q, k, v = split heads (H=12, HD=64); q *= HD**-0.5
    out = softmax(q k^T) v               # full [N, N] scores
    out = merge_heads(out) @ w_proj + b_proj

Distribution: pure data parallel over the batch dim — B=8 batch elements,
8 NeuronCores, one element per core.  Weights are replicated.  No
collectives are needed; each core computes its full [2048, 768] output.

Per-core compute strategy (all matmuls fp16, fp32 PSUM accumulation;
fp16 keeps 10 mantissa bits vs bf16's 7, tightening the error budget):
  * x is cast f32->fp16 by a SWDGE DMA into a DRAM scratch, then DMA-xbar
    transposed into SBUF as xT [768, 2048] in 512-token blocks.
  * qkT = w_qk^T @ x^T -> [1536, 2048]: q/k for a head PAIR live in one
    128-partition tile (head A on partitions 0-63, head B on 64-127).
  * scoresT[m, n] = kT^T qT per head: keys on partitions, queries on the
    free dim; one [128, 1024] PSUM tile per (pair, key-tile) covers both
    heads x 512 queries.
  * exp splits across two engines: most tiles on ScalarE (exp activation,
    softmax 1/8 scale folded into the free affine; no max subtraction -
    scaled scores are ~N(0,1)).  Key tiles in DVE_EXP_MS instead run a
    Schraudolph integer exp on VectorE: one tensor_scalar computes
    round(s*A + B) into an int16 alias of the fp16 tile, which IS the
    fp16 bit pattern of 2^(s*SCALE/ln2) (+-3% on those weights; washes
    out across the 2048-key softmax).
  * attnV runs in the natural (queries-on-partition) orientation with at
    as the stationary operand: out[128q, 64] = at_slice^T @ v_h, so each
    accumulation step streams only 64 columns - half the PE cost of the
    transposed orientation.  A companion 1-column ones matmul per slice
    accumulates the softmax denominator at ~zero PE cost.
  * normalization is a per-partition (per-query) reciprocal multiply -
    one [128,1] reciprocal + one tensor_scalar per (head, query-slice).
  * out_nat [512, 768] chunks are PE-transposed (identity matmul) into
    outT as soon as each head pair is evicted, feeding the final
    projection outT^T @ w_proj + b, interleaved into the next chunk.
"""

import os

import numpy as np

import concourse.bass as bass
import concourse.mybir as mybir
from concourse import bacc, bass_utils
from concourse.tile import TileContext

F32 = mybir.dt.float32
FP16 = mybir.dt.float16
I16 = mybir.dt.int16
AF = mybir.ActivationFunctionType

B, N, C = 8, 2048, 768
H, HD = 12, 64
SCALE = HD ** -0.5  # folded into the exp activation
P = 128
NT = N // P          # 16 key tiles
CT = C // P          # 6 feature tiles
NCHUNK = 4           # query chunks of 512
QW = N // NCHUNK     # 512
PAIRS = H // 2       # 6 head pairs
SKEW = 5             # scores/exp run this many key-tiles ahead of attnV

# Schraudolph integer exp: fp16 bits of 2^(s*SCALE/ln2) ~= round(s*A + B)
EXP_A = SCALE * 1024.0 / np.log(2.0)
EXP_B = 15.0 * 1024.0 - 44.0        # mid-centered: +-3% rel err
# key tiles exp'd on VectorE, per chunk: chunk 0 is PE/DVE-heavy (qkv
# projections + evictions) so ScalarE takes more of the exp there
DVE_EXP_MS = {0: (2, 6, 10, 14), 1: (2, 4, 7, 9, 12, 14),
              2: (2, 4, 7, 9, 12, 14), 3: (2, 4, 7, 9, 12, 14)}


def build_nc() -> bass.Bass:
    nc = bacc.Bacc(None)
    x = nc.declare_dram_parameter("x", [N, C], F32, isOutput=False)
    w_qkv = nc.declare_dram_parameter("w_qkv", [C, 3 * C], F32, isOutput=False)
    w_proj = nc.declare_dram_parameter("w_proj", [C, C], F32, isOutput=False)
    b_proj = nc.declare_dram_parameter("b_proj", [C], F32, isOutput=False)
    out = nc.declare_dram_parameter("out", [N, C], F32, isOutput=True)

    with TileContext(nc) as tc:
        with (
            tc.tile_pool(name="const", bufs=1) as cpool,
            tc.tile_pool(name="dram", bufs=1, space="DRAM") as dpool,
            tc.tile_pool(name="at", bufs=8) as at_pool,
            tc.tile_pool(name="rcp", bufs=8) as rcp_pool,
            tc.tile_pool(name="onat", bufs=2) as onat_pool,
            tc.tile_pool(name="fin", bufs=2) as fin_pool,
            tc.tile_pool(name="psc", bufs=2, space="PSUM") as psum_sc,
            tc.tile_pool(name="pqk", bufs=2, space="PSUM") as psum_qk,
            tc.tile_pool(name="pav", bufs=1, space="PSUM") as psum_av,
            tc.tile_pool(name="pden", bufs=1, space="PSUM") as psum_den,
        ):
            # ---- persistent SBUF tensors -------------------------------
            w_qkv_sb = cpool.tile([P, CT, 3 * C], FP16, tag="wqkv")
            wproj_sb = cpool.tile([P, CT, C], FP16, tag="wproj")
            b_bc = cpool.tile([P, C], F32, tag="bias")  # bias bcast to 128 rows
            onescol = cpool.tile([P, 1], FP16, tag="onescol")
            ident = cpool.tile([P, P], FP16, tag="ident")
            # 24 separate xT blocks (ct, token-chunk): independent tiles let
            # the 24 DMA transposes run without false write-order deps
            xT = {
                (ct, tch): cpool.tile(
                    [P, QW], FP16, tag=f"xT{ct}_{tch}", name=f"xT{ct}_{tch}"
                )
                for ct in range(CT)
                for tch in range(NCHUNK)
            }
            qkT = cpool.tile([P, 12, N], FP16, tag="qkT")  # q(6 pairs)+k(6)
            v4 = cpool.tile([P, NT, C], FP16, tag="v4")
            outT = cpool.tile([P, CT, N], FP16, tag="outT")

            # ---- phase 0: load + cast + transpose ----------------------
            # per-token-chunk contiguous f32->fp16 casts (cheap flat APs),
            # spread across engine DMA queues: chunk 0 on Pool (first), the
            # rest on ScalarE (idle until the first exp); the 24 xbar
            # transposes alternate between the SP and DVE queues
            nc.any.memset(onescol[:], 1.0)
            nc.any.memset(ident[:], 1.0)
            # identity: keep 1.0 where (p - col) == 0, else 0
            nc.gpsimd.affine_select(
                ident[:], ident[:], pattern=[[-1, P]],
                compare_op=mybir.AluOpType.is_equal, fill=0.0,
                base=0, channel_multiplier=1,
            )
            wq3 = w_qkv.rearrange("(o p) j -> p o j", p=P)
            wp3 = w_proj.rearrange("(o p) j -> p o j", p=P)
            # all casting DMAs must go via gpsimd; keep their APs 2-D
            # (column-split, non-contiguous) so each is charged as a short
            # per-row transfer.  Load order on the Pool queue: x chunk 0,
            # the k-weight block (first scores need it), remaining x chunks,
            # then q weights, v weights, w_proj.
            xblks = []
            for tch in range(NCHUNK):
                xblk = dpool.tile([QW, C], FP16, name=f"xf{tch}", tag=f"xf{tch}")
                xblks.append(xblk)

            def cast_x(tch):
                tsl = slice(tch * QW, (tch + 1) * QW)
                for half in range(2):
                    csl = slice(half * 384, (half + 1) * 384)
                    nc.gpsimd.dma_start(
                        out=xblks[tch][:, csl], in_=x[tsl, csl]
                    )

            def emit_transposes(tch):
                for ct in range(CT):
                    csl = slice(ct * P, (ct + 1) * P)
                    eng = nc.sync if (tch * CT + ct) % 2 == 0 else nc.scalar
                    eng.dma_start_transpose(
                        xT[(ct, tch)][:], xblks[tch][:, csl]
                    )

            def load_w_block(blk):  # 0=q, 1=k, 2=v columns of w_qkv
                jsl = slice(blk * C, (blk + 1) * C)
                for ct in range(CT):
                    nc.gpsimd.dma_start(
                        out=w_qkv_sb[:, ct, jsl], in_=wq3[:, ct, jsl]
                    )

            cast_x(0)
            emit_transposes(0)
            load_w_block(1)  # k weights
            load_w_block(0)  # q weights
            for tch in range(1, NCHUNK):
                cast_x(tch)
                emit_transposes(tch)
            load_w_block(2)  # v weights
            for ct in range(CT):
                nc.gpsimd.dma_start(out=wproj_sb[:, ct, :], in_=wp3[:, ct, :])
            nc.sync.dma_start(
                out=b_bc[:], in_=b_proj[None, :].to_broadcast((P, C))
            )

            # ---- qkv projection groups --------------------------------
            # qkT[j, n] for j in [0, 1536): q rows 0-767, k rows 768-1535
            def emit_qk_group(jt: int, c4: int):
                ps = psum_qk.tile([P, 512], F32, tag="qk", name="qkps")
                for ct in range(CT):
                    nc.tensor.matmul(
                        ps[:],
                        lhsT=w_qkv_sb[:, ct, jt * P : (jt + 1) * P],
                        rhs=xT[(ct, c4)][:],
                        start=(ct == 0),
                        stop=(ct == CT - 1),
                    )
                nc.vector.tensor_copy(
                    out=qkT[:, jt, c4 * QW : (c4 + 1) * QW], in_=ps[:]
                )

            # v natural layout: v[n, e] = sum_c x[n, c] w_qkv[c, 1536 + e]
            def emit_v_group(nt: int, hq: int):
                ps = psum_qk.tile([P, 512], F32, tag="qk", name="vps")
                eo = hq * 256
                for ct in range(CT):
                    nc.tensor.matmul(
                        ps[:, 0:256],
                        lhsT=xT[(ct, nt // 4)][:, (nt % 4) * P : (nt % 4 + 1) * P],
                        rhs=w_qkv_sb[:, ct, 2 * C + eo : 2 * C + eo + 256],
                        start=(ct == 0),
                        stop=(ct == CT - 1),
                    )
                nc.vector.tensor_copy(
                    out=v4[:, nt, eo : eo + 256], in_=ps[:, 0:256]
                )

            # ---- transposed eviction + final projection ----------------
            def emit_transpose_round(c_done: int, ct: int, onat):
                """outT[:, ct, c*512:+512] = out_nat chunk columns ct, via PE."""
                ps = psum_qk.tile([P, 512], FP16, tag="qk", name="trps")
                for qs in range(4):
                    nc.tensor.transpose(
                        ps[:, qs * P : (qs + 1) * P],
                        onat[:, qs, ct * P : (ct + 1) * P],
                        ident[:],
                    )
                nc.vector.tensor_copy(
                    out=outT[:, ct, c_done * QW : (c_done + 1) * QW], in_=ps[:]
                )

            def emit_proj_group(nt: int, eo: int, ew: int):
                """final[nt-tile, eo:eo+ew] = outT^T w_proj + b."""
                ps = psum_qk.tile([P, 512], F32, tag="qk", name="projps")
                for ct in range(CT):
                    nc.tensor.matmul(
                        ps[:, 0:ew],
                        lhsT=outT[:, ct, nt * P : (nt + 1) * P],
                        rhs=wproj_sb[:, ct, eo : eo + ew],
                        start=(ct == 0),
                        stop=(ct == CT - 1),
                    )
                fs = fin_pool.tile([P, 512], F32, tag="fin")
                nc.vector.tensor_tensor(
                    fs[:, 0:ew], ps[:, 0:ew], b_bc[:, eo : eo + ew],
                    mybir.AluOpType.add,
                )
                nc.sync.dma_start(
                    out=out[nt * P : (nt + 1) * P, eo : eo + ew], in_=fs[:, 0:ew]
                )

            def emit_proj_slot(c_done: int, slot: int):
                nt = c_done * 4 + slot // 2
                eo, ew = ((0, 512), (512, 256))[slot % 2]
                emit_proj_group(nt, eo, ew)

            # chunk-0 JIT schedule: (pair, scores-m) -> qk group
            c0_qk = {}
            for p0 in range(PAIRS):
                if p0 == 0:
                    c0_qk.update({(0, 2): (6, 1), (0, 5): (6, 2), (0, 8): (6, 3),
                                  (0, 10): (7, 0), (0, 11): (7, 1), (0, 12): (7, 2),
                                  (0, 13): (7, 3), (0, 14): (1, 0)})
                elif p0 < PAIRS - 1:
                    c0_qk.update({
                        (p0, 1): (7 + p0, 0), (p0, 4): (7 + p0, 1),
                        (p0, 7): (7 + p0, 2), (p0, 10): (7 + p0, 3),
                        (p0, 13): (1 + p0, 0),
                    })
                    if p0 == 1:
                        c0_qk[(p0, 14)] = (0, 1)
            # chunk 1 also absorbs chunk 0's deferred transposes (chunk 0's
            # own psum ring is saturated by the qkv projection storm)
            late_slots_c1 = {}
            for p0 in range(3):
                late_slots_c1[(p0, 5)] = ("t", 2 * p0)
                late_slots_c1[(p0, 11)] = ("t", 2 * p0 + 1)
            for p0 in range(3, PAIRS):
                late_slots_c1[(p0, 5)] = ("p", 2 * (p0 - 3))
                late_slots_c1[(p0, 11)] = ("p", 2 * (p0 - 3) + 1)
            late_slots_c1[(4, 13)] = ("p", 6)
            late_slots_c1[(5, 13)] = ("p", 7)
            # chunks >=2: proj of chunk c-1 spread over pairs 0-3
            late_slots = {}
            for p0 in range(4):
                late_slots[(p0, 5)] = ("p", 2 * p0)
                late_slots[(p0, 11)] = ("p", 2 * p0 + 1)

            # upfront: only what (c=0, pair=0, m<4) needs
            emit_qk_group(6, 0)   # kT pair 0, keys 0-511
            emit_qk_group(0, 0)   # qT pair 0, queries 0-511

            onat_prev = [None]  # chunk 0's onat, for its deferred transposes

            # ---- phase 2+3: attention + projection ---------------------
            for c in range(NCHUNK):
                qsl = slice(c * QW, (c + 1) * QW)
                onat = onat_pool.tile([P, 4, C], FP16, tag="onat", name="onat")
                for pair in range(PAIRS):
                    av = psum_av.tile([P, 512], F32, tag="av", name="av")
                    den = psum_den.tile([P, 8], F32, tag="den", name="den")
                    ats = [None] * NT
                    for mm_i in range(NT + SKEW):
                        # ---- scores + exp stage (runs SKEW ahead) ------
                        m = mm_i
                        if m < NT:
                            if c == 0:
                                if pair % 2 == 0:
                                    emit_v_group(m, pair // 2)
                                if (pair, m) in c0_qk:
                                    emit_qk_group(*c0_qk[(pair, m)])
                            else:
                                slots = late_slots_c1 if c == 1 else late_slots
                                if (pair, m) in slots:
                                    kind, s = slots[(pair, m)]
                                    if kind == "t":
                                        emit_transpose_round(0, s, onat_prev[0])
                                    else:
                                        emit_proj_slot(c - 1, s)
                                if c == 1 and m == 8 and pair < PAIRS - 1:
                                    emit_qk_group(pair + 1, 1)
                                if c < NCHUNK - 1 and m == 14:
                                    emit_qk_group(pair, c + 1)
                            msl = slice(m * P, (m + 1) * P)
                            sc = psum_sc.tile([P, 1024], F32, tag="sc")
                            nc.tensor.matmul(
                                sc[:, 0:QW],
                                lhsT=qkT[0:64, 6 + pair, msl],
                                rhs=qkT[0:64, pair, qsl],
                                start=True,
                                stop=True,
                            )
                            nc.tensor.matmul(
                                sc[:, QW : 2 * QW],
                                lhsT=qkT[64:128, 6 + pair, msl],
                                rhs=qkT[64:128, pair, qsl],
                                start=True,
                                stop=True,
                            )
                            at = at_pool.tile([P, 1024], FP16, tag="at")
                            ats[m] = at
                            if m in DVE_EXP_MS[c]:
                                # Schraudolph: int16 bits = round(s*A + B) are
                                # the fp16 pattern of exp(s*SCALE) (+-3%)
                                nc.vector.tensor_scalar(
                                    at.bitcast(I16)[:],
                                    sc[:],
                                    EXP_A,
                                    EXP_B,
                                    mybir.AluOpType.mult,
                                    mybir.AluOpType.add,
                                )
                            else:
                                nc.scalar.activation(
                                    at[:], sc[:], AF.Exp, scale=SCALE
                                )
                        # ---- attnV + denominator stage -----------------
                        if mm_i >= SKEW:
                            m = mm_i - SKEW
                            at = ats[m]
                            for hh in range(2):
                                h = 2 * pair + hh
                                for qs in range(4):
                                    lsl = at[:, hh * QW + qs * P : hh * QW + (qs + 1) * P]
                                    # start=True zeroes the whole 2KB bank
                                    # region, so only the bank's FIRST
                                    # sub-column may set it; later columns
                                    # overwrite-on-first-touch via the
                                    # pending-zero bytes it marked
                                    nc.tensor.matmul(
                                        av[:, (hh * 4 + qs) * 64 : (hh * 4 + qs + 1) * 64],
                                        lhsT=lsl,
                                        rhs=v4[:, m, h * 64 : (h + 1) * 64],
                                        start=(m == 0 and hh * 4 + qs == 0),
                                        stop=(m == NT - 1),
                                        skip_group_check=(hh * 4 + qs != 0),
                                    )
                                    nc.tensor.matmul(
                                        den[:, hh * 4 + qs : hh * 4 + qs + 1],
                                        lhsT=lsl,
                                        rhs=onescol[:],
                                        start=(m == 0 and hh * 4 + qs == 0),
                                        stop=(m == NT - 1),
                                        skip_group_check=(hh * 4 + qs != 0),
                                    )

                    # ---- normalize + evict this (pair, chunk) ----------
                    # reciprocal on DVE, per-partition-scaled Copy on ScalarE
                    for hh in range(2):
                        h = 2 * pair + hh
                        for qs in range(4):
                            rcp = rcp_pool.tile([P, 1], F32, tag="rcp")
                            nc.vector.reciprocal(
                                rcp[:], den[:, hh * 4 + qs : hh * 4 + qs + 1]
                            )
                            if hh == 0:
                                nc.vector.tensor_scalar(
                                    onat[:, qs, h * 64 : (h + 1) * 64],
                                    av[:, (hh * 4 + qs) * 64 : (hh * 4 + qs + 1) * 64],
                                    rcp[:],
                                    None,
                                    mybir.AluOpType.mult,
                                )
                            else:
                                nc.scalar.activation(
                                    onat[:, qs, h * 64 : (h + 1) * 64],
                                    av[:, (hh * 4 + qs) * 64 : (hh * 4 + qs + 1) * 64],
                                    AF.Copy,
                                    scale=rcp[:],
                                )
                    # transpose round `pair` only reads this pair's columns
                    # of onat (heads 2p, 2p+1 = feature block ct=pair), so it
                    # can run as soon as this pair is evicted (chunk 0's
                    # rounds are deferred into chunk 1 instead)
                    if c == 0:
                        onat_prev[0] = onat
                    else:
                        emit_transpose_round(c, pair, onat)
            # tail: proj for the last chunk
            for slot in range(8):
                emit_proj_slot(NCHUNK - 1, slot)

    nc.compile()
    return nc


_NC_CACHE: list = []


def _get_nc() -> bass.Bass:
    if not _NC_CACHE:
        _NC_CACHE.append(build_nc())
    return _NC_CACHE[0]


def run(inputs: dict, trace: bool = False):
    """Run on 8 NeuronCores.  Returns (out [B,N,C] f32, exec_time_ns|None)."""
    nc = _get_nc()
    x = np.ascontiguousarray(np.asarray(inputs["x"], dtype=np.float32))
    w_qkv = np.ascontiguousarray(np.asarray(inputs["w_qkv"], dtype=np.float32))
    w_proj = np.ascontiguousarray(np.asarray(inputs["w_proj"], dtype=np.float32))
    b_proj = np.ascontiguousarray(np.asarray(inputs["b_proj"], dtype=np.float32))
    in_maps = [
        {"x": x[i], "w_qkv": w_qkv, "w_proj": w_proj, "b_proj": b_proj}
        for i in range(B)
    ]
    try:
        res = bass_utils.run_bass_kernel_spmd(
            nc, in_maps, core_ids=list(range(B)), trace=trace
        )
    except ModuleNotFoundError:
        # NTFF profile hook unavailable in this image; run without trace
        res = bass_utils.run_bass_kernel_spmd(
            nc, in_maps, core_ids=list(range(B)), trace=False
        )
    out = np.stack([res.results[i]["out"] for i in range(B)], axis=0)
    return out.astype(np.float32), res.exec_time_ns


def kernel(x, w_qkv, w_proj, b_proj):
    trace = os.environ.get("BASS_KERNEL_TRACE", "0") == "1"
    out, _ = run(
        {"x": x, "w_qkv": w_qkv, "w_proj": w_proj, "b_proj": b_proj}, trace=trace
    )
    return out


# revision 44
# speedup vs baseline: 1.0530x; 1.0136x over previous
"""Trainium2 Bass kernel for multi-head attention (dense transformer block).

Reference computation (per batch element):
    qkv = x @ w_qkv                      # [N, 3C]
    q, k, v = split heads (H=12, HD=64); q *= HD**-0.5
    out = softmax(q k^T) v               # full [N, N] scores
    out = merge_heads(out) @ w_proj + b_proj

Distribution: pure data parallel over the batch dim — B=8 batch elements,
8 NeuronCores, one element per core.  Weights are replicated.  No
collectives are needed; each core computes its full [2048, 768] output.

Per-core compute strategy (all matmuls fp16, fp32 PSUM accumulation;
fp16 keeps 10 mantissa bits vs bf16's 7, tightening the error budget):
  * x is cast f32->fp16 by a SWDGE DMA into a DRAM scratch, then DMA-xbar
    transposed into SBUF as xT [768, 2048] in 512-token blocks.
  * qkT = w_qk^T @ x^T -> [1536, 2048]: q/k for a head PAIR live in one
    128-partition tile (head A on partitions 0-63, head B on 64-127).
  * scoresT[m, n] = kT^T qT per head: keys on partitions, queries on the
    free dim; one [128, 1024] PSUM tile per (pair, key-tile) covers both
    heads x 512 queries.
  * exp splits across two engines: most tiles on ScalarE (exp activation,
    softmax 1/8 scale folded into the free affine; no max subtraction -
    scaled scores are ~N(0,1)).  Key tiles in DVE_EXP_MS instead run a
    Schraudolph integer exp on VectorE: one tensor_scalar computes
    round(s*A + B) into an int16 alias of the fp16 tile, which IS the
    fp16 bit pattern of 2^(s*SCALE/ln2) (+-3% on those weights; washes
    out across the 2048-key softmax).
  * attnV runs in the natural (queries-on-partition) orientation with at
    as the stationary operand: out[128q, 64] = at_slice^T @ v_h, so each
    accumulation step streams only 64 columns - half the PE cost of the
    transposed orientation.  A companion 1-column ones matmul per slice
    accumulates the softmax denominator at ~zero PE cost.
  * normalization is a per-partition (per-query) reciprocal multiply -
    one [128,1] reciprocal + one tensor_scalar per (head, query-slice).
  * out_nat [512, 768] chunks are PE-transposed (identity matmul) into
    outT as soon as each head pair is evicted, feeding the final
    projection outT^T @ w_proj + b, interleaved into the next chunk.
"""

import os

import numpy as np

import concourse.bass as bass
import concourse.mybir as mybir
from concourse import bacc, bass_utils
from concourse.tile import TileContext

F32 = mybir.dt.float32
FP16 = mybir.dt.float16
I16 = mybir.dt.int16
AF = mybir.ActivationFunctionType

B, N, C = 8, 2048, 768
H, HD = 12, 64
SCALE = HD ** -0.5  # folded into the exp activation
P = 128
NT = N // P          # 16 key tiles
CT = C // P          # 6 feature tiles
NCHUNK = 4           # query chunks of 512
QW = N // NCHUNK     # 512
PAIRS = H // 2       # 6 head pairs
SKEW = 6             # scores/exp run this many key-tiles ahead of attnV

# Schraudolph integer exp: fp16 bits of 2^(s*SCALE/ln2) ~= round(s*A + B)
EXP_A = SCALE * 1024.0 / np.log(2.0)
EXP_B = 15.0 * 1024.0 - 44.0        # mid-centered: +-3% rel err
# key tiles exp'd on VectorE, per chunk: chunk 0 is PE/DVE-heavy (qkv
# projections + evictions) so ScalarE takes more of the exp there
DVE_EXP_MS = {0: (2, 6, 10, 14), 1: (2, 4, 7, 9, 12, 14),
              2: (2, 4, 7, 9, 12, 14), 3: (2, 4, 7, 9, 12, 14)}


def build_nc() -> bass.Bass:
    nc = bacc.Bacc(None)
    x = nc.declare_dram_parameter("x", [N, C], F32, isOutput=False)
    w_qkv = nc.declare_dram_parameter("w_qkv", [C, 3 * C], F32, isOutput=False)
    w_proj = nc.declare_dram_parameter("w_proj", [C, C], F32, isOutput=False)
    b_proj = nc.declare_dram_parameter("b_proj", [C], F32, isOutput=False)
    out = nc.declare_dram_parameter("out", [N, C], F32, isOutput=True)

    with TileContext(nc) as tc:
        with (
            tc.tile_pool(name="const", bufs=1) as cpool,
            tc.tile_pool(name="dram", bufs=1, space="DRAM") as dpool,
            tc.tile_pool(name="at", bufs=8) as at_pool,
            tc.tile_pool(name="rcp", bufs=8) as rcp_pool,
            tc.tile_pool(name="onat", bufs=2) as onat_pool,
            tc.tile_pool(name="fin", bufs=2) as fin_pool,
            tc.tile_pool(name="psc", bufs=2, space="PSUM") as psum_sc,
            tc.tile_pool(name="pqk", bufs=2, space="PSUM") as psum_qk,
            tc.tile_pool(name="pav", bufs=1, space="PSUM") as psum_av,
            tc.tile_pool(name="pden", bufs=1, space="PSUM") as psum_den,
        ):
            # ---- persistent SBUF tensors -------------------------------
            w_qkv_sb = cpool.tile([P, CT, 3 * C], FP16, tag="wqkv")
            wproj_sb = cpool.tile([P, CT, C], FP16, tag="wproj")
            b_bc = cpool.tile([P, C], F32, tag="bias")  # bias bcast to 128 rows
            onescol = cpool.tile([P, 1], FP16, tag="onescol")
            ident = cpool.tile([P, P], FP16, tag="ident")
            # 24 separate xT blocks (ct, token-chunk): independent tiles let
            # the 24 DMA transposes run without false write-order deps
            xT = {
                (ct, tch): cpool.tile(
                    [P, QW], FP16, tag=f"xT{ct}_{tch}", name=f"xT{ct}_{tch}"
                )
                for ct in range(CT)
                for tch in range(NCHUNK)
            }
            qkT = cpool.tile([P, 12, N], FP16, tag="qkT")  # q(6 pairs)+k(6)
            v4 = cpool.tile([P, NT, C], FP16, tag="v4")
            outT = cpool.tile([P, CT, N], FP16, tag="outT")

            # ---- phase 0: load + cast + transpose ----------------------
            # per-token-chunk contiguous f32->fp16 casts (cheap flat APs),
            # spread across engine DMA queues: chunk 0 on Pool (first), the
            # rest on ScalarE (idle until the first exp); the 24 xbar
            # transposes alternate between the SP and DVE queues
            nc.any.memset(onescol[:], 1.0)
            nc.any.memset(ident[:], 1.0)
            # identity: keep 1.0 where (p - col) == 0, else 0
            nc.gpsimd.affine_select(
                ident[:], ident[:], pattern=[[-1, P]],
                compare_op=mybir.AluOpType.is_equal, fill=0.0,
                base=0, channel_multiplier=1,
            )
            wq3 = w_qkv.rearrange("(o p) j -> p o j", p=P)
            wp3 = w_proj.rearrange("(o p) j -> p o j", p=P)
            # all casting DMAs must go via gpsimd; keep their APs 2-D
            # (column-split, non-contiguous) so each is charged as a short
            # per-row transfer.  Load order on the Pool queue: x chunk 0,
            # the k-weight block (first scores need it), remaining x chunks,
            # then q weights, v weights, w_proj.
            xblks = []
            for tch in range(NCHUNK):
                xblk = dpool.tile([QW, C], FP16, name=f"xf{tch}", tag=f"xf{tch}")
                xblks.append(xblk)

            def cast_x(tch):
                tsl = slice(tch * QW, (tch + 1) * QW)
                for half in range(2):
                    csl = slice(half * 384, (half + 1) * 384)
                    nc.gpsimd.dma_start(
                        out=xblks[tch][:, csl], in_=x[tsl, csl]
                    )

            def emit_transposes(tch):
                for ct in range(CT):
                    csl = slice(ct * P, (ct + 1) * P)
                    eng = nc.sync if (tch * CT + ct) % 2 == 0 else nc.scalar
                    eng.dma_start_transpose(
                        xT[(ct, tch)][:], xblks[tch][:, csl]
                    )

            def load_w_block(blk):  # 0=q, 1=k, 2=v columns of w_qkv
                jsl = slice(blk * C, (blk + 1) * C)
                for ct in range(CT):
                    nc.gpsimd.dma_start(
                        out=w_qkv_sb[:, ct, jsl], in_=wq3[:, ct, jsl]
                    )

            cast_x(0)
            emit_transposes(0)
            load_w_block(1)  # k weights
            load_w_block(0)  # q weights
            for tch in range(1, NCHUNK):
                cast_x(tch)
                emit_transposes(tch)
            load_w_block(2)  # v weights
            for ct in range(CT):
                nc.gpsimd.dma_start(out=wproj_sb[:, ct, :], in_=wp3[:, ct, :])
            nc.sync.dma_start(
                out=b_bc[:], in_=b_proj[None, :].to_broadcast((P, C))
            )

            # ---- qkv projection groups --------------------------------
            # qkT[j, n] for j in [0, 1536): q rows 0-767, k rows 768-1535
            def emit_qk_group(jt: int, c4: int):
                ps = psum_qk.tile([P, 512], F32, tag="qk", name="qkps")
                for ct in range(CT):
                    nc.tensor.matmul(
                        ps[:],
                        lhsT=w_qkv_sb[:, ct, jt * P : (jt + 1) * P],
                        rhs=xT[(ct, c4)][:],
                        start=(ct == 0),
                        stop=(ct == CT - 1),
                    )
                nc.vector.tensor_copy(
                    out=qkT[:, jt, c4 * QW : (c4 + 1) * QW], in_=ps[:]
                )

            # v natural layout: v[n, e] = sum_c x[n, c] w_qkv[c, 1536 + e]
            def emit_v_group(nt: int, hq: int):
                ps = psum_qk.tile([P, 512], F32, tag="qk", name="vps")
                eo = hq * 256
                for ct in range(CT):
                    nc.tensor.matmul(
                        ps[:, 0:256],
                        lhsT=xT[(ct, nt // 4)][:, (nt % 4) * P : (nt % 4 + 1) * P],
                        rhs=w_qkv_sb[:, ct, 2 * C + eo : 2 * C + eo + 256],
                        start=(ct == 0),
                        stop=(ct == CT - 1),
                    )
                nc.vector.tensor_copy(
                    out=v4[:, nt, eo : eo + 256], in_=ps[:, 0:256]
                )

            # ---- transposed eviction + final projection ----------------
            def emit_transpose_round(c_done: int, ct: int, onat):
                """outT[:, ct, c*512:+512] = out_nat chunk columns ct, via PE."""
                ps = psum_qk.tile([P, 512], FP16, tag="qk", name="trps")
                for qs in range(4):
                    nc.tensor.transpose(
                        ps[:, qs * P : (qs + 1) * P],
                        onat[:, qs, ct * P : (ct + 1) * P],
                        ident[:],
                    )
                nc.vector.tensor_copy(
                    out=outT[:, ct, c_done * QW : (c_done + 1) * QW], in_=ps[:]
                )

            def emit_proj_group(nt: int, eo: int, ew: int):
                """final[nt-tile, eo:eo+ew] = outT^T w_proj + b."""
                ps = psum_qk.tile([P, 512], F32, tag="qk", name="projps")
                for ct in range(CT):
                    nc.tensor.matmul(
                        ps[:, 0:ew],
                        lhsT=outT[:, ct, nt * P : (nt + 1) * P],
                        rhs=wproj_sb[:, ct, eo : eo + ew],
                        start=(ct == 0),
                        stop=(ct == CT - 1),
                    )
                fs = fin_pool.tile([P, 512], F32, tag="fin")
                nc.vector.tensor_tensor(
                    fs[:, 0:ew], ps[:, 0:ew], b_bc[:, eo : eo + ew],
                    mybir.AluOpType.add,
                )
                nc.sync.dma_start(
                    out=out[nt * P : (nt + 1) * P, eo : eo + ew], in_=fs[:, 0:ew]
                )

            def emit_proj_slot(c_done: int, slot: int):
                nt = c_done * 4 + slot // 2
                eo, ew = ((0, 512), (512, 256))[slot % 2]
                emit_proj_group(nt, eo, ew)

            # chunk-0 JIT schedule: (pair, scores-m) -> qk group
            c0_qk = {}
            for p0 in range(PAIRS):
                if p0 == 0:
                    c0_qk.update({(0, 2): (6, 1), (0, 5): (6, 2), (0, 8): (6, 3),
                                  (0, 10): (7, 0), (0, 11): (7, 1), (0, 12): (7, 2),
                                  (0, 13): (7, 3), (0, 14): (1, 0)})
                elif p0 < PAIRS - 1:
                    c0_qk.update({
                        (p0, 1): (7 + p0, 0), (p0, 4): (7 + p0, 1),
                        (p0, 7): (7 + p0, 2), (p0, 10): (7 + p0, 3),
                        (p0, 13): (1 + p0, 0),
                    })
                    if p0 == 1:
                        c0_qk[(p0, 14)] = (0, 1)
            # chunk 1 also absorbs chunk 0's deferred transposes (chunk 0's
            # own psum ring is saturated by the qkv projection storm)
            late_slots_c1 = {}
            for p0 in range(3):
                late_slots_c1[(p0, 5)] = ("t", 2 * p0)
                late_slots_c1[(p0, 11)] = ("t", 2 * p0 + 1)
            for p0 in range(3, PAIRS):
                late_slots_c1[(p0, 5)] = ("p", 2 * (p0 - 3))
                late_slots_c1[(p0, 11)] = ("p", 2 * (p0 - 3) + 1)
            late_slots_c1[(4, 13)] = ("p", 6)
            late_slots_c1[(5, 13)] = ("p", 7)
            # chunks >=2: proj of chunk c-1 spread over pairs 0-3
            late_slots = {}
            for p0 in range(4):
                late_slots[(p0, 5)] = ("p", 2 * p0)
                late_slots[(p0, 11)] = ("p", 2 * p0 + 1)

            # upfront: only what (c=0, pair=0, m<4) needs
            emit_qk_group(6, 0)   # kT pair 0, keys 0-511
            emit_qk_group(0, 0)   # qT pair 0, queries 0-511

            onat_prev = [None]  # chunk 0's onat, for its deferred transposes

            # ---- phase 2+3: attention + projection ---------------------
            for c in range(NCHUNK):
                qsl = slice(c * QW, (c + 1) * QW)
                onat = onat_pool.tile([P, 4, C], FP16, tag="onat", name="onat")
                for pair in range(PAIRS):
                    av = psum_av.tile([P, 512], F32, tag="av", name="av")
                    den = psum_den.tile([P, 8], F32, tag="den", name="den")
                    ats = [None] * NT
                    for mm_i in range(NT + SKEW):
                        # ---- scores + exp stage (runs SKEW ahead) ------
                        m = mm_i
                        if m < NT:
                            if c == 0:
                                if pair % 2 == 0:
                                    emit_v_group(m, pair // 2)
                                if (pair, m) in c0_qk:
                                    emit_qk_group(*c0_qk[(pair, m)])
                            else:
                                slots = late_slots_c1 if c == 1 else late_slots
                                if (pair, m) in slots:
                                    kind, s = slots[(pair, m)]
                                    if kind == "t":
                                        emit_transpose_round(0, s, onat_prev[0])
                                    else:
                                        emit_proj_slot(c - 1, s)
                                if c == 1 and m == 8 and pair < PAIRS - 1:
                                    emit_qk_group(pair + 1, 1)
                                if c < NCHUNK - 1 and m == 14:
                                    emit_qk_group(pair, c + 1)
                            msl = slice(m * P, (m + 1) * P)
                            sc = psum_sc.tile([P, 1024], F32, tag="sc")
                            nc.tensor.matmul(
                                sc[:, 0:QW],
                                lhsT=qkT[0:64, 6 + pair, msl],
                                rhs=qkT[0:64, pair, qsl],
                                start=True,
                                stop=True,
                            )
                            nc.tensor.matmul(
                                sc[:, QW : 2 * QW],
                                lhsT=qkT[64:128, 6 + pair, msl],
                                rhs=qkT[64:128, pair, qsl],
                                start=True,
                                stop=True,
                            )
                            at = at_pool.tile([P, 1024], FP16, tag="at")
                            ats[m] = at
                            if m in DVE_EXP_MS[c]:
                                # Schraudolph: int16 bits = round(s*A + B) are
                                # the fp16 pattern of exp(s*SCALE) (+-3%)
                                nc.vector.tensor_scalar(
                                    at.bitcast(I16)[:],
                                    sc[:],
                                    EXP_A,
                                    EXP_B,
                                    mybir.AluOpType.mult,
                                    mybir.AluOpType.add,
                                )
                            else:
                                nc.scalar.activation(
                                    at[:], sc[:], AF.Exp, scale=SCALE
                                )
                        # ---- attnV + denominator stage -----------------
                        if mm_i >= SKEW:
                            m = mm_i - SKEW
                            at = ats[m]
                            for hh in range(2):
                                h = 2 * pair + hh
                                for qs in range(4):
                                    lsl = at[:, hh * QW + qs * P : hh * QW + (qs + 1) * P]
                                    # start=True zeroes the whole 2KB bank
                                    # region, so only the bank's FIRST
                                    # sub-column may set it; later columns
                                    # overwrite-on-first-touch via the
                                    # pending-zero bytes it marked
                                    nc.tensor.matmul(
                                        av[:, (hh * 4 + qs) * 64 : (hh * 4 + qs + 1) * 64],
                                        lhsT=lsl,
                                        rhs=v4[:, m, h * 64 : (h + 1) * 64],
                                        start=(m == 0 and hh * 4 + qs == 0),
                                        stop=(m == NT - 1),
                                        skip_group_check=(hh * 4 + qs != 0),
                                    )
                                    nc.tensor.matmul(
                                        den[:, hh * 4 + qs : hh * 4 + qs + 1],
                                        lhsT=lsl,
                                        rhs=onescol[:],
                                        start=(m == 0 and hh * 4 + qs == 0),
                                        stop=(m == NT - 1),
                                        skip_group_check=(hh * 4 + qs != 0),
                                    )

                    # ---- normalize + evict this (pair, chunk) ----------
                    # all 8 reciprocals first so the den bank frees early
                    # for the next pair's accumulation start
                    rcps = []
                    for k8 in range(8):
                        rcp = rcp_pool.tile([P, 1], F32, tag="rcp")
                        rcps.append(rcp)
                        nc.vector.reciprocal(rcp[:], den[:, k8 : k8 + 1])
                    for hh in range(2):
                        h = 2 * pair + hh
                        for qs in range(4):
                            nc.vector.tensor_scalar(
                                onat[:, qs, h * 64 : (h + 1) * 64],
                                av[:, (hh * 4 + qs) * 64 : (hh * 4 + qs + 1) * 64],
                                rcps[hh * 4 + qs][:],
                                None,
                                mybir.AluOpType.mult,
                            )
                    # transpose round `pair` only reads this pair's columns
                    # of onat (heads 2p, 2p+1 = feature block ct=pair), so it
                    # can run as soon as this pair is evicted (chunk 0's
                    # rounds are deferred into chunk 1 instead)
                    if c == 0:
                        onat_prev[0] = onat
                    else:
                        emit_transpose_round(c, pair, onat)
            # tail: proj for the last chunk
            for slot in range(8):
                emit_proj_slot(NCHUNK - 1, slot)

    nc.compile()
    return nc


_NC_CACHE: list = []


def _get_nc() -> bass.Bass:
    if not _NC_CACHE:
        _NC_CACHE.append(build_nc())
    return _NC_CACHE[0]


def run(inputs: dict, trace: bool = False):
    """Run on 8 NeuronCores.  Returns (out [B,N,C] f32, exec_time_ns|None)."""
    nc = _get_nc()
    x = np.ascontiguousarray(np.asarray(inputs["x"], dtype=np.float32))
    w_qkv = np.ascontiguousarray(np.asarray(inputs["w_qkv"], dtype=np.float32))
    w_proj = np.ascontiguousarray(np.asarray(inputs["w_proj"], dtype=np.float32))
    b_proj = np.ascontiguousarray(np.asarray(inputs["b_proj"], dtype=np.float32))
    in_maps = [
        {"x": x[i], "w_qkv": w_qkv, "w_proj": w_proj, "b_proj": b_proj}
        for i in range(B)
    ]
    try:
        res = bass_utils.run_bass_kernel_spmd(
            nc, in_maps, core_ids=list(range(B)), trace=trace
        )
    except ModuleNotFoundError:
        # NTFF profile hook unavailable in this image; run without trace
        res = bass_utils.run_bass_kernel_spmd(
            nc, in_maps, core_ids=list(range(B)), trace=False
        )
    out = np.stack([res.results[i]["out"] for i in range(B)], axis=0)
    return out.astype(np.float32), res.exec_time_ns


def kernel(x, w_qkv, w_proj, b_proj):
    trace = os.environ.get("BASS_KERNEL_TRACE", "0") == "1"
    out, _ = run(
        {"x": x, "w_qkv": w_qkv, "w_proj": w_proj, "b_proj": b_proj}, trace=trace
    )
    return out
